# revision 12
# baseline (speedup 1.0000x reference)
"""Blockwise 8x8 2D orthonormal DCT (Dct2d) for Trainium2, 8 NeuronCores.

Input  x: (64, 1, 1024, 1024) f32  ->  Output: (64, 64, 128, 128) f32
Data parallel over the batch dim: 8 samples per core.

Per-core algorithm (per 128-row strip of each 1024x1024 image):
  load: Pool-engine (SWDGE) DMA casts x f32 -> bf16 in flight, so the
      SBUF-side transfer is half the bytes and no separate cast op runs.
  mm1 (per 128-col tile t): PSUM[w, (gh,i)] = X_t^T @ C,  C = I_16 (x) A^T
      (data tile is the *stationary* operand, so the transpose is fused;
      bf16 operands run the PE at 1 cycle/row instead of fp32's 4)
  mm2 (per tile t):         PSUM[(gh,i), (j,gw16)] = Y1_t^T @ R,
      R[(g,l),(j,g)] = A[j,l]  (permuted block-diagonal)
  strided PSUM->SBUF copies assemble [(gh,i), j, gw] so the HBM store has
  contiguous 512B runs per (channel, row).  Stores are issued from the SP
  queue, loads from Pool, so neither blocks the other's issue pipeline.
"""

from contextlib import ExitStack

import ml_dtypes
import numpy as np

import concourse.bass as bass
import concourse.tile as tile
from concourse import bacc, mybir
from concourse.bass_utils import run_bass_kernel_spmd

N_CORES = 8
H = W = 1024
N_STRIPS = H // 128  # 8
STRIPS_PER_LOAD = 2  # strips per casting DMA (256 descs); bufs bound in-flight descs


def _dct_consts(A: np.ndarray) -> tuple[np.ndarray, np.ndarray]:
    A = np.asarray(A, np.float32)
    C = np.zeros((128, 128), np.float32)
    R = np.zeros((128, 128), np.float32)
    for g in range(16):
        C[g * 8 : (g + 1) * 8, g * 8 : (g + 1) * 8] = A.T
    for g in range(16):
        for l in range(8):
            for j in range(8):
                R[g * 8 + l, j * 16 + g] = A[j, l]
    return C, R


def _build(samples: int, Cmat: np.ndarray, Rmat: np.ndarray) -> bass.Bass:
    nc = bacc.Bacc(
        "TRN2", target_bir_lowering=False, debug=False, num_devices=N_CORES
    )
    f32 = mybir.dt.float32
    bf16 = mybir.dt.bfloat16
    x_ap = nc.dram_tensor("x", (samples, H, W), f32, kind="ExternalInput").ap()
    out_ap = nc.dram_tensor(
        "out", (samples, 64, H // 8, W // 8), f32, kind="ExternalOutput"
    ).ap()
    cd = nc.inline_tensor(
        Cmat.astype(ml_dtypes.bfloat16), name="cmat"
    ).ap()
    rd = nc.inline_tensor(
        Rmat.astype(ml_dtypes.bfloat16), name="rmat"
    ).ap()

    with tile.TileContext(nc) as tc, ExitStack() as ctx:
        consts = ctx.enter_context(tc.tile_pool(name="consts", bufs=1))
        xpool = ctx.enter_context(tc.tile_pool(name="xs", bufs=3))
        y1pool = ctx.enter_context(tc.tile_pool(name="y1", bufs=4))
        opool = ctx.enter_context(tc.tile_pool(name="os", bufs=4))
        ps1 = ctx.enter_context(tc.tile_pool(name="ps1", bufs=4, space="PSUM"))
        ps2 = ctx.enter_context(tc.tile_pool(name="ps2", bufs=4, space="PSUM"))

        ct = consts.tile([128, 128], bf16)
        nc.sync.dma_start(ct[:], cd[:])
        rt = consts.tile([128, 128], bf16)
        nc.sync.dma_start(rt[:], rd[:])

        for s in range(samples):
            spl = STRIPS_PER_LOAD
            pool_s = xpool
            for chunk in range(N_STRIPS // spl):
                # f32 -> bf16 casting load (software DGE on the Pool engine)
                xt = pool_s.tile([128, spl, 1024], bf16)
                src = x_ap[
                    s, chunk * spl * 128 : (chunk + 1) * spl * 128, :
                ].rearrange("(t p) c -> p t c", p=128)
                nc.gpsimd.dma_start(xt[:], src)

                for st4 in range(spl):
                    st = chunk * spl + st4
                    # columns t*128 + (gh*8+i): row-DCT'd, transposed tiles
                    y1 = y1pool.tile([128, 1024], bf16)
                    for b in range(2):
                        p1 = ps1.tile([128, 512], f32)
                        for t4 in range(4):
                            t = b * 4 + t4
                            nc.tensor.matmul(
                                p1[:, t4 * 128 : (t4 + 1) * 128],
                                lhsT=xt[:, st4, t * 128 : (t + 1) * 128],
                                rhs=ct[:],
                                start=(t4 == 0),
                                stop=(t4 == 3),
                            )
                        if b == 0:
                            nc.vector.tensor_copy(y1[:, 0:512], p1[:])
                        else:
                            nc.scalar.copy(y1[:, 512:1024], p1[:])

                    # [p=(gh,i), j, gw]
                    ot = opool.tile([128, 8, 128], f32)
                    for b in range(2):
                        p2 = ps2.tile([128, 512], f32)
                        for t4 in range(4):
                            t = b * 4 + t4
                            nc.tensor.matmul(
                                p2[:, t4 * 128 : (t4 + 1) * 128],
                                lhsT=y1[:, t * 128 : (t + 1) * 128],
                                rhs=rt[:],
                                start=(t4 == 0),
                                stop=(t4 == 3),
                            )
                        # psum col (t4, j, g) -> ot[:, j, b*64 + t4*16 + g]
                        src2 = p2.rearrange("p (t j g) -> p t j g", t=4, j=8)
                        dst = ot[:, :, b * 64 : (b + 1) * 64].rearrange(
                            "p j (t g) -> p t j g", t=4
                        )
                        if b == 0:
                            nc.scalar.copy(dst, src2)
                        else:
                            nc.vector.tensor_copy(dst, src2)

                    dram_view = out_ap[
                        s, :, st * 16 : (st + 1) * 16, :
                    ].rearrange("(i j) gh gw -> gh i j gw", i=8)
                    nc.sync.dma_start(dram_view, ot[:])

    nc.compile()
    return nc


_cache: dict = {}


def _get_program(samples: int, A: np.ndarray) -> bass.Bass:
    key = (samples, A.tobytes())
    if key not in _cache:
        C, R = _dct_consts(A)
        _cache[key] = _build(samples, C, R)
    return _cache[key]


def _run(x, A, **spmd_kwargs):
    x = np.ascontiguousarray(np.asarray(x, dtype=np.float32))
    A = np.asarray(A, dtype=np.float32)
    N = x.shape[0]
    spc = N // N_CORES  # samples per core
    nc = _get_program(spc, A)
    in_maps = [
        {"x": np.ascontiguousarray(x[i * spc : (i + 1) * spc, 0])}
        for i in range(N_CORES)
    ]
    res = run_bass_kernel_spmd(nc, in_maps, list(range(N_CORES)), **spmd_kwargs)
    out = np.concatenate(
        [res.results[i]["out"] for i in range(N_CORES)], axis=0
    )
    return out.astype(np.float32, copy=False), res


def kernel(x, A):
    out, _ = _run(x, A)
    return out


# revision 14
# speedup vs baseline: 1.0002x; 1.0002x over previous
"""Blockwise 8x8 2D orthonormal DCT (Dct2d) for Trainium2, 8 NeuronCores.

Input  x: (64, 1, 1024, 1024) f32  ->  Output: (64, 64, 128, 128) f32
Data parallel over the batch dim: 8 samples per core.

Per-core algorithm (per 128-row strip of each 1024x1024 image):
  load: Pool-engine (SWDGE) DMA casts x f32 -> bf16 in flight, so the
      SBUF-side transfer is half the bytes and no separate cast op runs.
  mm1 (per 128-col tile t): PSUM[w, (gh,i)] = X_t^T @ C,  C = I_16 (x) A^T
      (data tile is the *stationary* operand, so the transpose is fused;
      bf16 operands run the PE at 1 cycle/row instead of fp32's 4)
  mm2 (per tile t):         PSUM[(gh,i), (j,gw16)] = Y1_t^T @ R,
      R[(g,l),(j,g)] = A[j,l]  (permuted block-diagonal)
  strided PSUM->SBUF copies assemble [(gh,i), j, gw] so the HBM store has
  contiguous 512B runs per (channel, row).  Stores are issued from the SP
  queue, loads from Pool, so neither blocks the other's issue pipeline.
"""

from contextlib import ExitStack

import ml_dtypes
import numpy as np

import concourse.bass as bass
import concourse.tile as tile
from concourse import bacc, mybir
from concourse.bass_utils import run_bass_kernel_spmd

N_CORES = 8
H = W = 1024
N_STRIPS = H // 128  # 8
STRIPS_PER_LOAD = 1  # one casting DMA per strip (128 descs); bufs bound in-flight descs


def _dct_consts(A: np.ndarray) -> tuple[np.ndarray, np.ndarray]:
    A = np.asarray(A, np.float32)
    C = np.zeros((128, 128), np.float32)
    R = np.zeros((128, 128), np.float32)
    for g in range(16):
        C[g * 8 : (g + 1) * 8, g * 8 : (g + 1) * 8] = A.T
    for g in range(16):
        for l in range(8):
            for j in range(8):
                R[g * 8 + l, j * 16 + g] = A[j, l]
    return C, R


def _build(samples: int, Cmat: np.ndarray, Rmat: np.ndarray) -> bass.Bass:
    nc = bacc.Bacc(
        "TRN2", target_bir_lowering=False, debug=False, num_devices=N_CORES
    )
    f32 = mybir.dt.float32
    bf16 = mybir.dt.bfloat16
    x_ap = nc.dram_tensor("x", (samples, H, W), f32, kind="ExternalInput").ap()
    out_ap = nc.dram_tensor(
        "out", (samples, 64, H // 8, W // 8), f32, kind="ExternalOutput"
    ).ap()
    cd = nc.inline_tensor(
        Cmat.astype(ml_dtypes.bfloat16), name="cmat"
    ).ap()
    rd = nc.inline_tensor(
        Rmat.astype(ml_dtypes.bfloat16), name="rmat"
    ).ap()

    with tile.TileContext(nc) as tc, ExitStack() as ctx:
        consts = ctx.enter_context(tc.tile_pool(name="consts", bufs=1))
        xpool = ctx.enter_context(tc.tile_pool(name="xs", bufs=6))
        y1pool = ctx.enter_context(tc.tile_pool(name="y1", bufs=4))
        opool = ctx.enter_context(tc.tile_pool(name="os", bufs=4))
        ps1 = ctx.enter_context(tc.tile_pool(name="ps1", bufs=3, space="PSUM"))
        ps2 = ctx.enter_context(tc.tile_pool(name="ps2", bufs=3, space="PSUM"))
        wpool = ctx.enter_context(tc.tile_pool(name="wps", bufs=1, space="PSUM"))

        ct = consts.tile([128, 128], bf16)
        nc.sync.dma_start(ct[:], cd[:])
        rt = consts.tile([128, 128], bf16)
        nc.sync.dma_start(rt[:], rd[:])

        warm = consts.tile([128, 256], bf16)
        nc.vector.memset(warm[:], 0.0)
        wps = wpool.tile([128, 256], f32)
        for _ in range(16):
            nc.tensor.matmul(
                wps[:], lhsT=warm[:, 0:128], rhs=warm[:], start=True, stop=True
            )

        for s in range(samples):
            spl = STRIPS_PER_LOAD
            pool_s = xpool
            for chunk in range(N_STRIPS // spl):
                # f32 -> bf16 casting load (software DGE on the Pool engine)
                xt = pool_s.tile([128, spl, 1024], bf16)
                src = x_ap[
                    s, chunk * spl * 128 : (chunk + 1) * spl * 128, :
                ].rearrange("(t p) c -> p t c", p=128)
                nc.gpsimd.dma_start(xt[:], src)

                for st4 in range(spl):
                    st = chunk * spl + st4
                    # columns t*128 + (gh*8+i): row-DCT'd, transposed tiles
                    y1 = y1pool.tile([128, 1024], bf16)
                    for b in range(2):
                        p1 = ps1.tile([128, 512], f32)
                        for t4 in range(4):
                            t = b * 4 + t4
                            nc.tensor.matmul(
                                p1[:, t4 * 128 : (t4 + 1) * 128],
                                lhsT=xt[:, st4, t * 128 : (t + 1) * 128],
                                rhs=ct[:],
                                start=(t4 == 0),
                                stop=(t4 == 3),
                            )
                        if b == 0:
                            nc.vector.tensor_copy(y1[:, 0:512], p1[:])
                        else:
                            nc.scalar.copy(y1[:, 512:1024], p1[:])

                    # [p=(gh,i), j, gw]
                    ot = opool.tile([128, 8, 128], f32)
                    for b in range(2):
                        p2 = ps2.tile([128, 512], f32)
                        for t4 in range(4):
                            t = b * 4 + t4
                            nc.tensor.matmul(
                                p2[:, t4 * 128 : (t4 + 1) * 128],
                                lhsT=y1[:, t * 128 : (t + 1) * 128],
                                rhs=rt[:],
                                start=(t4 == 0),
                                stop=(t4 == 3),
                            )
                        # psum col (t4, j, g) -> ot[:, j, b*64 + t4*16 + g]
                        src2 = p2.rearrange("p (t j g) -> p t j g", t=4, j=8)
                        dst = ot[:, :, b * 64 : (b + 1) * 64].rearrange(
                            "p j (t g) -> p t j g", t=4
                        )
                        # split each PSUM->SBUF assemble across both engines
                        if b == 0:
                            nc.scalar.copy(dst[:, 0:2], src2[:, 0:2])
                            nc.vector.tensor_copy(dst[:, 2:4], src2[:, 2:4])
                        else:
                            nc.vector.tensor_copy(dst[:, 0:2], src2[:, 0:2])
                            nc.scalar.copy(dst[:, 2:4], src2[:, 2:4])

                    dram_view = out_ap[
                        s, :, st * 16 : (st + 1) * 16, :
                    ].rearrange("(i j) gh gw -> gh i j gw", i=8)
                    nc.sync.dma_start(dram_view, ot[:])

    nc.compile()
    return nc


_cache: dict = {}


def _get_program(samples: int, A: np.ndarray) -> bass.Bass:
    key = (samples, A.tobytes())
    if key not in _cache:
        C, R = _dct_consts(A)
        _cache[key] = _build(samples, C, R)
    return _cache[key]


def _run(x, A, **spmd_kwargs):
    x = np.ascontiguousarray(np.asarray(x, dtype=np.float32))
    A = np.asarray(A, dtype=np.float32)
    N = x.shape[0]
    spc = N // N_CORES  # samples per core
    nc = _get_program(spc, A)
    in_maps = [
        {"x": np.ascontiguousarray(x[i * spc : (i + 1) * spc, 0])}
        for i in range(N_CORES)
    ]
    res = run_bass_kernel_spmd(nc, in_maps, list(range(N_CORES)), **spmd_kwargs)
    out = np.concatenate(
        [res.results[i]["out"] for i in range(N_CORES)], axis=0
    )
    return out.astype(np.float32, copy=False), res


def kernel(x, A):
    out, _ = _run(x, A)
    return out


# revision 21
# speedup vs baseline: 1.0280x; 1.0278x over previous
"""Blockwise 8x8 2D orthonormal DCT (Dct2d) for Trainium2, 8 NeuronCores.

Input  x: (64, 1, 1024, 1024) f32  ->  Output: (64, 64, 128, 128) f32
Data parallel over the batch dim: 8 samples per core.

Per-core algorithm (per 128-row strip of each 1024x1024 image):
  load: Pool-engine (SWDGE) DMA casts x f32 -> bf16 in flight, so the
      SBUF-side transfer is half the bytes and no separate cast op runs.
  mm1 (per 128-col tile t): PSUM[w, (gh,i)] = X_t^T @ C,  C = I_16 (x) A^T
      (data tile is the *stationary* operand, so the transpose is fused;
      bf16 operands run the PE at 1 cycle/row instead of fp32's 4)
  mm2 (per tile t):         PSUM[(gh,i), (j,gw16)] = Y1_t^T @ R,
      R[(g,l),(j,g)] = A[j,l]  (permuted block-diagonal)
  strided PSUM->SBUF copies assemble [(gh,i), j, gw] so the HBM store has
  contiguous 512B runs per (channel, row).  Stores are issued from the SP
  queue, loads from Pool, so neither blocks the other's issue pipeline.
"""

from contextlib import ExitStack

import ml_dtypes
import numpy as np

import concourse.bass as bass
import concourse.tile as tile
from concourse import bacc, mybir
from concourse.bass_utils import run_bass_kernel_spmd

N_CORES = 8
H = W = 1024
N_STRIPS = H // 128  # 8
STRIPS_PER_LOAD = 1  # one casting DMA per strip (128 descs); bufs bound in-flight descs


def _dct_consts(A: np.ndarray) -> tuple[np.ndarray, np.ndarray]:
    A = np.asarray(A, np.float32)
    C = np.zeros((128, 128), np.float32)
    R = np.zeros((128, 128), np.float32)
    for g in range(16):
        C[g * 8 : (g + 1) * 8, g * 8 : (g + 1) * 8] = A.T
    for g in range(16):
        for l in range(8):
            for j in range(8):
                R[g * 8 + l, j * 16 + g] = A[j, l]
    return C, R


def _build(samples: int, Cmat: np.ndarray, Rmat: np.ndarray) -> bass.Bass:
    nc = bacc.Bacc(
        "TRN2", target_bir_lowering=False, debug=False, num_devices=N_CORES
    )
    f32 = mybir.dt.float32
    bf16 = mybir.dt.bfloat16
    x_ap = nc.dram_tensor("x", (samples, H, W), f32, kind="ExternalInput").ap()
    out_ap = nc.dram_tensor(
        "out", (samples, 64, H // 8, W // 8), f32, kind="ExternalOutput"
    ).ap()
    cd = nc.inline_tensor(
        Cmat.astype(ml_dtypes.bfloat16), name="cmat"
    ).ap()
    rd = nc.inline_tensor(
        Rmat.astype(ml_dtypes.bfloat16), name="rmat"
    ).ap()

    with tile.TileContext(nc) as tc, ExitStack() as ctx:
        consts = ctx.enter_context(tc.tile_pool(name="consts", bufs=1))
        xpool = ctx.enter_context(tc.tile_pool(name="xs", bufs=6))
        y1pool = ctx.enter_context(tc.tile_pool(name="y1", bufs=4))
        opool = ctx.enter_context(tc.tile_pool(name="os", bufs=6))
        ps1 = ctx.enter_context(tc.tile_pool(name="ps1", bufs=3, space="PSUM"))
        ps2 = ctx.enter_context(tc.tile_pool(name="ps2", bufs=3, space="PSUM"))
        wpool = ctx.enter_context(tc.tile_pool(name="wps", bufs=1, space="PSUM"))

        ct = consts.tile([128, 128], bf16)
        nc.sync.dma_start(ct[:], cd[:])
        rt = consts.tile([128, 128], bf16)
        nc.sync.dma_start(rt[:], rd[:])

        # PE warm-up: keep the tensor engine busy from t~0 so the first real
        # matmuls run at full clock (p-state ramps with continuous activity)
        warm = consts.tile([128, 256], bf16)
        nc.vector.memset(warm[:], 0.0)
        wps = wpool.tile([128, 512], f32)
        for _ in range(12):
            nc.tensor.matmul(
                wps[:, 0:256], lhsT=warm[:, 0:128], rhs=warm[:],
                start=True, stop=True,
            )

        for s in range(samples):
            spl = STRIPS_PER_LOAD
            pool_s = xpool
            for chunk in range(N_STRIPS // spl):
                # f32 -> bf16 casting load (software DGE on the Pool engine)
                xt = pool_s.tile([128, spl, 1024], bf16)
                src = x_ap[
                    s, chunk * spl * 128 : (chunk + 1) * spl * 128, :
                ].rearrange("(t p) c -> p t c", p=128)
                nc.gpsimd.dma_start(xt[:], src)

                for st4 in range(spl):
                    st = chunk * spl + st4
                    # columns t*128 + (gh*8+i): row-DCT'd, transposed tiles
                    y1 = y1pool.tile([128, 1024], bf16)
                    for b in range(2):
                        p1 = ps1.tile([128, 512], f32)
                        for t4 in range(4):
                            t = b * 4 + t4
                            nc.tensor.matmul(
                                p1[:, t4 * 128 : (t4 + 1) * 128],
                                lhsT=xt[:, st4, t * 128 : (t + 1) * 128],
                                rhs=ct[:],
                                start=(t4 == 0),
                                stop=(t4 == 3),
                            )
                        # first two strips: split copies across engines to
                        # reach the first store sooner (fill-phase latency)
                        fastlane = s == 0 and st < 2
                        if b == 0:
                            if fastlane:
                                nc.vector.tensor_copy(y1[:, 0:256], p1[:, 0:256])
                                nc.scalar.copy(y1[:, 256:512], p1[:, 256:512])
                            else:
                                nc.vector.tensor_copy(y1[:, 0:512], p1[:])
                        else:
                            if fastlane:
                                nc.scalar.copy(y1[:, 512:768], p1[:, 0:256])
                                nc.vector.tensor_copy(y1[:, 768:1024], p1[:, 256:512])
                            else:
                                nc.scalar.copy(y1[:, 512:1024], p1[:])

                    # [p=(gh,i), j, gw]
                    ot = opool.tile([128, 8, 128], f32)
                    for b in range(2):
                        p2 = ps2.tile([128, 512], f32)
                        for t4 in range(4):
                            t = b * 4 + t4
                            nc.tensor.matmul(
                                p2[:, t4 * 128 : (t4 + 1) * 128],
                                lhsT=y1[:, t * 128 : (t + 1) * 128],
                                rhs=rt[:],
                                start=(t4 == 0),
                                stop=(t4 == 3),
                            )
                        # psum col (t4, j, g) -> ot[:, j, b*64 + t4*16 + g]
                        src2 = p2.rearrange("p (t j g) -> p t j g", t=4, j=8)
                        dst = ot[:, :, b * 64 : (b + 1) * 64].rearrange(
                            "p j (t g) -> p t j g", t=4
                        )
                        if b == 0:
                            nc.scalar.copy(dst, src2)
                        else:
                            nc.vector.tensor_copy(dst, src2)

                    dram_view = out_ap[
                        s, :, st * 16 : (st + 1) * 16, :
                    ].rearrange("(i j) gh gw -> gh i j gw", i=8)
                    nc.sync.dma_start(dram_view, ot[:])

    nc.compile()
    return nc


_cache: dict = {}


def _get_program(samples: int, A: np.ndarray) -> bass.Bass:
    key = (samples, A.tobytes())
    if key not in _cache:
        C, R = _dct_consts(A)
        _cache[key] = _build(samples, C, R)
    return _cache[key]


def _run(x, A, **spmd_kwargs):
    x = np.ascontiguousarray(np.asarray(x, dtype=np.float32))
    A = np.asarray(A, dtype=np.float32)
    N = x.shape[0]
    spc = N // N_CORES  # samples per core
    nc = _get_program(spc, A)
    in_maps = [
        {"x": np.ascontiguousarray(x[i * spc : (i + 1) * spc, 0])}
        for i in range(N_CORES)
    ]
    res = run_bass_kernel_spmd(nc, in_maps, list(range(N_CORES)), **spmd_kwargs)
    out = np.concatenate(
        [res.results[i]["out"] for i in range(N_CORES)], axis=0
    )
    return out.astype(np.float32, copy=False), res


def kernel(x, A):
    out, _ = _run(x, A)
    return out


# revision 22
# speedup vs baseline: 1.0301x; 1.0021x over previous
"""Blockwise 8x8 2D orthonormal DCT (Dct2d) for Trainium2, 8 NeuronCores.

Input  x: (64, 1, 1024, 1024) f32  ->  Output: (64, 64, 128, 128) f32
Data parallel over the batch dim: 8 samples per core.

Per-core algorithm (per 128-row strip of each 1024x1024 image):
  load: Pool-engine (SWDGE) DMA casts x f32 -> bf16 in flight, so the
      SBUF-side transfer is half the bytes and no separate cast op runs.
  mm1 (per 128-col tile t): PSUM[w, (gh,i)] = X_t^T @ C,  C = I_16 (x) A^T
      (data tile is the *stationary* operand, so the transpose is fused;
      bf16 operands run the PE at 1 cycle/row instead of fp32's 4)
  mm2 (per tile t):         PSUM[(gh,i), (j,gw16)] = Y1_t^T @ R,
      R[(g,l),(j,g)] = A[j,l]  (permuted block-diagonal)
  strided PSUM->SBUF copies assemble [(gh,i), j, gw] so the HBM store has
  contiguous 512B runs per (channel, row).  Stores are issued from the SP
  queue, loads from Pool, so neither blocks the other's issue pipeline.
"""

from contextlib import ExitStack

import ml_dtypes
import numpy as np

import concourse.bass as bass
import concourse.tile as tile
from concourse import bacc, mybir
from concourse.bass_utils import run_bass_kernel_spmd

N_CORES = 8
H = W = 1024
N_STRIPS = H // 128  # 8
STRIPS_PER_LOAD = 1  # one casting DMA per strip (128 descs); bufs bound in-flight descs


def _dct_consts(A: np.ndarray) -> tuple[np.ndarray, np.ndarray]:
    A = np.asarray(A, np.float32)
    C = np.zeros((128, 128), np.float32)
    R = np.zeros((128, 128), np.float32)
    for g in range(16):
        C[g * 8 : (g + 1) * 8, g * 8 : (g + 1) * 8] = A.T
    for g in range(16):
        for l in range(8):
            for j in range(8):
                R[g * 8 + l, j * 16 + g] = A[j, l]
    return C, R


def _build(samples: int, Cmat: np.ndarray, Rmat: np.ndarray) -> bass.Bass:
    nc = bacc.Bacc(
        "TRN2", target_bir_lowering=False, debug=False, num_devices=N_CORES
    )
    f32 = mybir.dt.float32
    bf16 = mybir.dt.bfloat16
    x_ap = nc.dram_tensor("x", (samples, H, W), f32, kind="ExternalInput").ap()
    out_ap = nc.dram_tensor(
        "out", (samples, 64, H // 8, W // 8), f32, kind="ExternalOutput"
    ).ap()
    cd = nc.inline_tensor(
        Cmat.astype(ml_dtypes.bfloat16), name="cmat"
    ).ap()
    rd = nc.inline_tensor(
        Rmat.astype(ml_dtypes.bfloat16), name="rmat"
    ).ap()

    with tile.TileContext(nc) as tc, ExitStack() as ctx:
        consts = ctx.enter_context(tc.tile_pool(name="consts", bufs=1))
        xpool = ctx.enter_context(tc.tile_pool(name="xs", bufs=6))
        y1pool = ctx.enter_context(tc.tile_pool(name="y1", bufs=4))
        opool = ctx.enter_context(tc.tile_pool(name="os", bufs=6))
        ps1 = ctx.enter_context(tc.tile_pool(name="ps1", bufs=3, space="PSUM"))
        ps2 = ctx.enter_context(tc.tile_pool(name="ps2", bufs=3, space="PSUM"))
        wpool = ctx.enter_context(tc.tile_pool(name="wps", bufs=1, space="PSUM"))

        ct = consts.tile([128, 128], bf16)
        nc.sync.dma_start(ct[:], cd[:])
        rt = consts.tile([128, 128], bf16)
        nc.sync.dma_start(rt[:], rd[:])

        # PE warm-up: keep the tensor engine busy from t~0 so the first real
        # matmuls run at full clock (p-state ramps with continuous activity)
        warm = consts.tile([128, 256], bf16)
        nc.vector.memset(warm[:], 0.0)
        wps = wpool.tile([128, 512], f32)
        for _ in range(12):
            nc.tensor.matmul(
                wps[:, 0:256], lhsT=warm[:, 0:128], rhs=warm[:],
                start=True, stop=True,
            )

        # fill-phase gap filler: tiny stores to DRAM scratch become ready in
        # the 4-9us window where SWDGE desc-gen (994ns fixed) cannot feed the
        # DMA engines fast enough; 182ns each, they soak up the idle slots
        scratch = nc.dram_tensor(
            "scratch", (6, 128, 256), bf16, kind="Internal"
        ).ap()
        for i in range(6):
            nc.sync.dma_start(scratch[i], warm[:])

        for s in range(samples):
            spl = STRIPS_PER_LOAD
            pool_s = xpool
            for chunk in range(N_STRIPS // spl):
                # f32 -> bf16 casting load (software DGE on the Pool engine)
                xt = pool_s.tile([128, spl, 1024], bf16)
                src = x_ap[
                    s, chunk * spl * 128 : (chunk + 1) * spl * 128, :
                ].rearrange("(t p) c -> p t c", p=128)
                nc.gpsimd.dma_start(xt[:], src)

                for st4 in range(spl):
                    st = chunk * spl + st4
                    # columns t*128 + (gh*8+i): row-DCT'd, transposed tiles
                    y1 = y1pool.tile([128, 1024], bf16)
                    for b in range(2):
                        p1 = ps1.tile([128, 512], f32)
                        for t4 in range(4):
                            t = b * 4 + t4
                            nc.tensor.matmul(
                                p1[:, t4 * 128 : (t4 + 1) * 128],
                                lhsT=xt[:, st4, t * 128 : (t + 1) * 128],
                                rhs=ct[:],
                                start=(t4 == 0),
                                stop=(t4 == 3),
                            )
                        # first two strips: split copies across engines to
                        # reach the first store sooner (fill-phase latency)
                        fastlane = s == 0 and st < 2
                        if b == 0:
                            if fastlane:
                                nc.vector.tensor_copy(y1[:, 0:256], p1[:, 0:256])
                                nc.scalar.copy(y1[:, 256:512], p1[:, 256:512])
                            else:
                                nc.vector.tensor_copy(y1[:, 0:512], p1[:])
                        else:
                            if fastlane:
                                nc.scalar.copy(y1[:, 512:768], p1[:, 0:256])
                                nc.vector.tensor_copy(y1[:, 768:1024], p1[:, 256:512])
                            else:
                                nc.scalar.copy(y1[:, 512:1024], p1[:])

                    # [p=(gh,i), j, gw]
                    ot = opool.tile([128, 8, 128], f32)
                    for b in range(2):
                        p2 = ps2.tile([128, 512], f32)
                        for t4 in range(4):
                            t = b * 4 + t4
                            nc.tensor.matmul(
                                p2[:, t4 * 128 : (t4 + 1) * 128],
                                lhsT=y1[:, t * 128 : (t + 1) * 128],
                                rhs=rt[:],
                                start=(t4 == 0),
                                stop=(t4 == 3),
                            )
                        # psum col (t4, j, g) -> ot[:, j, b*64 + t4*16 + g]
                        src2 = p2.rearrange("p (t j g) -> p t j g", t=4, j=8)
                        dst = ot[:, :, b * 64 : (b + 1) * 64].rearrange(
                            "p j (t g) -> p t j g", t=4
                        )
                        if b == 0:
                            nc.scalar.copy(dst, src2)
                        else:
                            nc.vector.tensor_copy(dst, src2)

                    dram_view = out_ap[
                        s, :, st * 16 : (st + 1) * 16, :
                    ].rearrange("(i j) gh gw -> gh i j gw", i=8)
                    nc.sync.dma_start(dram_view, ot[:])

    nc.compile()
    return nc


_cache: dict = {}


def _get_program(samples: int, A: np.ndarray) -> bass.Bass:
    key = (samples, A.tobytes())
    if key not in _cache:
        C, R = _dct_consts(A)
        _cache[key] = _build(samples, C, R)
    return _cache[key]


def _run(x, A, **spmd_kwargs):
    x = np.ascontiguousarray(np.asarray(x, dtype=np.float32))
    A = np.asarray(A, dtype=np.float32)
    N = x.shape[0]
    spc = N // N_CORES  # samples per core
    nc = _get_program(spc, A)
    in_maps = [
        {"x": np.ascontiguousarray(x[i * spc : (i + 1) * spc, 0])}
        for i in range(N_CORES)
    ]
    res = run_bass_kernel_spmd(nc, in_maps, list(range(N_CORES)), **spmd_kwargs)
    out = np.concatenate(
        [res.results[i]["out"] for i in range(N_CORES)], axis=0
    )
    return out.astype(np.float32, copy=False), res


def kernel(x, A):
    out, _ = _run(x, A)
    return out
